# revision 20
# baseline (speedup 1.0000x reference)
"""Trainium2 Bass kernel for a ResNet Bottleneck block (inference).

Reference computation (NCHW, N=128, Cin=Cout=1024, width=256, H=W=14):
    out = relu(bn1(conv1x1(x, w1)))          # 1024 -> 256
    out = relu(bn2(conv3x3(out, w2, pad=1))) # 256 -> 256
    out = bn3(conv1x1(out, w3))              # 256 -> 1024
    y   = relu(out + x)

Strategy:
- Data-parallel: batch 128 sharded as 16 images per NeuronCore (8 cores),
  conv/BN params replicated. One NEFF, SPMD via run_bass_kernel_spmd.
- BN folded on host into per-channel weight scale + bias.
- All convs run in fp8-e4m3 with MatmulPerfMode.DoubleRow: each matmul
  contracts K=256 (two 128-channel blocks stacked in AP dim 1) at double
  the bf16 MAC rate. Weights/activations are scaled host-side
  (s1=16, s2=64, s3=256) to sit in e4m3's healthy range; the scale is
  unwound for free: relu(s*a) = s*relu(a), so each conv's input scale is
  folded into the next conv's weights, and the final 1/s3 rides the
  eviction op.
- fp32 PSUM accumulation. conv3's residual is added in PSUM by a bf16
  identity-weight matmul (weights = s3*I) on the bf16 x tiles, so conv3
  eviction is a single relu-and-scale op per group. conv3's BN bias is
  folded into the residual tiles host-side (x + b3).
- Per-image-pair layouts use row-interleaved fields (j = 2*row + img) so
  the 3x3 conv's DoubleRow moving operand is a 4-dim AP
  [p, kpair, 28 interleaved rows, 14 cols] over a zero-padded 32x16
  field.
- PSUM groups are allocated as 2-bank pair tiles [P, 1024] so evictions
  process two groups per DVE/ACT op (halves op count + semaphores).
"""

import sys

if "/opt/trn_rl_repo" not in sys.path:
    sys.path.insert(0, "/opt/trn_rl_repo")

import numpy as np
import ml_dtypes

import concourse.bass as bass
import concourse.bacc as bacc
import concourse.tile as tile
from concourse import mybir
from concourse.bass_utils import run_bass_kernel_spmd

EPS = 1e-5
NCORES = 8
NLOC = 16          # images per core
C_IN = 1024
WIDTH = 256
C_OUT = 1024
HW = 196           # 14*14
P = 128
KB1 = C_IN // P    # 8 input channel blocks
KP1 = KB1 // 2     # 4 DoubleRow channel-block pairs for conv1
KB2 = WIDTH // P   # 2 channel blocks for conv2/conv3 input
MB3 = C_OUT // P   # 8 output channel blocks for conv3
NPAIRS = NLOC // 2  # 8 image pairs; N=392 per matmul
NF = 2 * HW        # 392
FLD = 512          # padded interleaved pair-field: 32 rows x 16 cols
BANK = 512         # PSUM bank, fp32 elements per partition

S1, S2, S3 = 16.0, 64.0, 256.0

F8 = mybir.dt.float8e4
BF16 = mybir.dt.bfloat16
F32 = mybir.dt.float32
Relu = mybir.ActivationFunctionType.Relu
DR = mybir.MatmulPerfMode.DoubleRow

_cached = {}


def _build():
    """Build + compile the SPMD NEFF (one core's program). Cached."""
    if "nc" in _cached:
        return _cached["nc"]

    nc = bacc.Bacc("TRN2", target_bir_lowering=False, debug=False,
                   num_devices=NCORES)

    # x tensors are partition-major in DRAM: per partition one long
    # contiguous run per DMA chunk (best descriptor efficiency)
    xq_d = nc.dram_tensor("xq", [P, KB1 * NLOC * HW], F8,
                          kind="ExternalInput")
    xr_d = nc.dram_tensor("xr", [P, KB1 * NLOC * HW], BF16,
                          kind="ExternalInput")
    w1_d = nc.dram_tensor("w1t", [P, KP1 * 2 * WIDTH], F8,
                          kind="ExternalInput")
    w2_d = nc.dram_tensor("w2t", [P, 9 * KB2 * WIDTH], F8,
                          kind="ExternalInput")
    w3_d = nc.dram_tensor("w3t", [P, KB2 * C_OUT], F8, kind="ExternalInput")
    b_d = nc.dram_tensor("biases", [P, 2 * KB2], F32, kind="ExternalInput")
    id_d = nc.dram_tensor("ident", [P, P], BF16, kind="ExternalInput")
    y_d = nc.dram_tensor("y", [MB3, P, NLOC * HW], BF16, kind="ExternalOutput")

    with tile.TileContext(nc) as tc:
        _emit(tc, nc, xq_d, xr_d, w1_d, w2_d, w3_d, b_d, id_d, y_d)

    nc.compile()
    _cached["nc"] = nc
    return nc


def _emit(tc, nc, xq_d, xr_d, w1_d, w2_d, w3_d, b_d, id_d, y_d):
    """PE-density-oriented emission.

    - Every phase runs its contraction loop OUTER over 8 concurrently-open
      PSUM groups (8 banks via 4 two-bank pair tiles), group index
      innermost, so consecutive matmuls target different banks and
      pipeline at the issue rate (~165ns for N=392 fp8 DoubleRow).
    - Startup: the PE clock needs ~3us of continuous activity to reach
      2.4 GHz, so warm-up matmuls are gated only on a scratch memset that
      is the FIRST gpsimd instruction (before any DMA issue). The xq
      stream gets exclusive DMA bandwidth until it completes; xr/w2/w3
      are chained behind it.
    - Evictions alternate DVE/ACT, one op per PSUM-bank pair.
    """
    import contextlib

    Alu = mybir.AluOpType

    with contextlib.ExitStack() as ctx:
        const = ctx.enter_context(tc.tile_pool(name="const", bufs=1))
        xpool = ctx.enter_context(tc.tile_pool(name="xpool", bufs=1))
        opool = ctx.enter_context(tc.tile_pool(name="opool", bufs=1))
        psp = ctx.enter_context(tc.tile_pool(name="psp", bufs=8, space="PSUM"))
        evp = ctx.enter_context(tc.tile_pool(name="evp", bufs=2))

        from concourse.tile import add_dep_helper

        # PE warm-up first: scratch memset is gpsimd's first instruction,
        # so the dummy matmuls start right after the framework preamble
        # and keep the PE busy (ramping its clock) while x loads.
        # ---- Loads ------------------------------------------------------
        # Three independent hw DMA queues (one per issuing engine):
        # sync + gpsimd stripe the x tensors (both queues in parallel),
        # scalar carries all the small weight/bias loads.
        w1sb = const.tile([P, KP1 * 2 * WIDTH], F8, name="w1sb", tag="w1sb")
        nc.scalar.dma_start(w1sb[:], w1_d.ap())
        w1v = w1sb[:].rearrange("p (j i c) -> p j i c", j=KP1, i=2)

        ball = const.tile([P, 2 * KB2], F32, name="ball", tag="ball")
        nc.scalar.dma_start(ball[:], b_d.ap())
        b1_t = ball[:, 0:KB2]
        b2_t = ball[:, KB2:2 * KB2]

        w2sb = const.tile([P, 9 * KB2 * WIDTH], F8, name="w2sb", tag="w2sb")
        nc.scalar.dma_start(w2sb[:], w2_d.ap())
        w2v = w2sb[:].rearrange("p (t i c) -> p t i c", t=9, i=2)

        w3sb = const.tile([P, KB2 * C_OUT], F8, name="w3sb", tag="w3sb")
        nc.scalar.dma_start(w3sb[:], w3_d.ap())
        w3v = w3sb[:].rearrange("p (i c) -> p i c", i=2)

        id_t = const.tile([P, P], BF16, name="id_t", tag="id_t")
        nc.scalar.dma_start(id_t[:], id_d.ap())

        xq = xpool.tile([P, KB1 * NLOC * HW], F8, name="xq", tag="xq")
        xqv = xq[:].rearrange("p (k n) -> p k n", k=KB1)
        # 8 fine chunks (one k-block each) alternating queues so the first
        # DoubleRow pair is available as early as possible and supply
        # always leads conv1's consumption.
        CQ = NLOC * HW
        xq_dmas = []
        for j in range(KB1):
            eng = nc.sync if j % 2 == 0 else nc.gpsimd
            i = eng.dma_start(xq[:, j * CQ:(j + 1) * CQ],
                              xq_d.ap()[:, j * CQ:(j + 1) * CQ])
            n = len(xq_dmas)
            if n >= 2:
                add_dep_helper(i.ins, xq_dmas[n - 2], reason="xq load pacing")
            xq_dmas.append(i.ins)

        # residual x (bf16, conv3 bias folded in), behind xq on both queues
        xr = xpool.tile([P, KB1 * NLOC * HW], BF16, name="xr", tag="xr")
        xrv = xr[:].rearrange("p (k n) -> p k n", k=KB1)
        CH = 2 * NLOC * HW
        xr_dmas = []
        for j in range(KB1 // 2):
            eng = nc.sync if j % 2 == 0 else nc.gpsimd
            i = eng.dma_start(xr[:, j * CH:(j + 1) * CH],
                              xr_d.ap()[:, j * CH:(j + 1) * CH])
            n = len(xr_dmas)
            if n < 2:
                add_dep_helper(i.ins, xq_dmas[6 + n], reason="xr behind xq")
            else:
                add_dep_helper(i.ins, xr_dmas[n - 2], reason="xr load pacing")
            xr_dmas.append(i.ins)

        # conv1 output: zero-padded row-interleaved pair fields, fp8.
        # Per image pair a 32x16 field (j = 2*padrow + img), payload at
        # j in 2..29, cols 1..14. Layout [P, KB2 * NPAIRS * FLD].
        # Only the pad cells are zeroed (3 small memsets, not the full
        # field): top/bottom pad rows and the left/right pad columns.
        out1 = opool.tile([P, KB2 * NPAIRS * FLD], F8, name="out1",
                          tag="out1")
        kq = KB2 * NPAIRS  # 16 fields, stride FLD
        fv = out1[:].rearrange("p (f j c) -> p f j c", f=kq, j=32, c=16)
        nc.vector.memset(fv[:, :, 0:2, :], 0.0)      # top pad rows j=0,1
        nc.vector.memset(fv[:, :, 30:32, :], 0.0)    # bottom pad rows
        nc.vector.memset(fv[:, :, 2:30, 0:1], 0.0)   # left pad col
        nc.vector.memset(fv[:, :, 2:30, 15:16], 0.0)  # right pad col
        out1v = out1[:].rearrange("p (k q j c) -> p k q j c",
                                  k=KB2, q=NPAIRS, j=32, c=16)

        out2 = opool.tile([P, KB2 * NLOC * HW], F8, name="out2", tag="out2")
        out2v = out2[:].rearrange("p (k n) -> p k n", k=KB2)

        # ---- conv1 (1x1, 1024->256) + bias + relu -> padded out1 --------
        # Per np-half: 8 open groups (4 npairs x 2 m), contraction kp outer.
        for half in range(2):
            nps = [half * 4 + j for j in range(4)]
            grp = {}
            for np_ in nps:
                for m in range(KB2):
                    grp[(np_, m)] = psp.tile([P, NF], F32,
                                             name=f"ps1_{np_}_{m}", tag="ps")
            for kp in range(KP1):
                for m in range(KB2):
                    for np_ in nps:
                        nc.tensor.matmul(
                            grp[(np_, m)][:],
                            w1v[:, kp, :, m * P:(m + 1) * P],
                            xqv[:, 2 * kp:2 * kp + 2,
                                np_ * NF:(np_ + 1) * NF],
                            start=(kp == 0), stop=(kp == KP1 - 1),
                            perf_mode=DR,
                        )
            for np_ in nps:
                for m in range(KB2):
                    dst = out1v[:, m, np_, 2:30, 1:15]
                    src = (grp[(np_, m)][:]
                           .rearrange("p (j c) -> p j c", j=28))
                    if np_ % 2 == 1:
                        nc.vector.tensor_scalar(dst, src, b1_t[:, m:m + 1],
                                                0.0, Alu.add, Alu.max)
                    else:
                        nc.scalar.activation(dst, src, Relu,
                                             bias=b1_t[:, m:m + 1])

        # ---- conv2 (3x3, 256->256, pad 1) + bias + relu -> out2 ----------
        # Per np-half: 8 open groups, contraction tap outer (each tap is a
        # DoubleRow pair over the two input channel blocks). The moving
        # operand is the shifted window over the interleaved padded field:
        # rows 2*dy..2*dy+27, cols dx..dx+13.
        for half in range(2):
            nps = [half * 4 + j for j in range(4)]
            grp = {}
            for np_ in nps:
                for m in range(KB2):
                    grp[(np_, m)] = psp.tile([P, NF], F32,
                                             name=f"ps2_{np_}_{m}", tag="ps")
            for t in range(9):
                dy, dx = t // 3, t % 3
                for m in range(KB2):
                    for np_ in nps:
                        rhs = out1v[:, :, np_, 2 * dy:2 * dy + 28,
                                    dx:dx + 14]
                        nc.tensor.matmul(
                            grp[(np_, m)][:]
                            .rearrange("p (j c) -> p j c", j=28),
                            w2v[:, t, :, m * P:(m + 1) * P],
                            rhs,
                            start=(t == 0), stop=(t == 8),
                            perf_mode=DR,
                        )
            for np_ in nps:
                for m in range(KB2):
                    dst = out2v[:, m, np_ * NF:(np_ + 1) * NF]
                    src = grp[(np_, m)][:]
                    if np_ % 2 == 1:
                        nc.vector.tensor_scalar(dst, src, b2_t[:, m:m + 1],
                                                0.0, Alu.add, Alu.max)
                    else:
                        nc.scalar.activation(dst, src, Relu,
                                             bias=b2_t[:, m:m + 1])

        # ---- conv3 (1x1, 256->1024) + residual + relu -> y --------------
        # Per m: 8 open groups as 4 pair tiles. Each group is one DoubleRow
        # matmul (K=256) plus a bf16 identity matmul (weights s3*I) adding
        # s3 * (x + b3) into PSUM; eviction is relu(psum)/s3 per pair.
        inv_s3 = 1.0 / S3
        for m in range(MB3):
            grp = {}
            for np_ in range(NPAIRS):
                grp[np_] = psp.tile([P, NF], F32, name=f"ps3_{np_}", tag="ps")
            for np_ in range(NPAIRS):
                nc.tensor.matmul(
                    grp[np_][:],
                    w3v[:, :, m * P:(m + 1) * P],
                    out2v[:, :, np_ * NF:(np_ + 1) * NF],
                    start=True, stop=False,
                    perf_mode=DR,
                )
            for np_ in range(NPAIRS):
                nc.tensor.matmul(
                    grp[np_][:], id_t[:],
                    xrv[:, m, np_ * NF:(np_ + 1) * NF],
                    start=False, stop=True,
                )
            ystage = evp.tile([P, NLOC * HW], BF16, name="ystage",
                              tag="ystage", bufs=3)
            for np_ in range(NPAIRS):
                dst = ystage[:, np_ * NF:(np_ + 1) * NF]
                if (np_ + m) % 2 == 1:
                    nc.vector.tensor_scalar(dst, grp[np_][:], 0.0, inv_s3,
                                            Alu.max, Alu.mult)
                else:
                    nc.scalar.activation(dst, grp[np_][:], Relu,
                                         bias=0.0, scale=inv_s3)
            nchunk = 4 if m == MB3 - 1 else 2
            CNF = NLOC * HW // nchunk
            for c in range(nchunk):
                eng = nc.sync if (m * 2 + c) % 2 == 0 else nc.gpsimd
                eng.dma_start(y_d.ap()[m][:, c * CNF:(c + 1) * CNF],
                              ystage[:, c * CNF:(c + 1) * CNF])


def _prep(x, w1, g1, b1, m1, v1, w2, g2, b2, m2, v2, w3, g3, b3, m3, v3):
    """Host-side: fold BN, scale + quantize weights to fp8, arrange SBUF
    images, shard + interleave x."""
    def fold(w, g, b, m, v):
        scale = (g.astype(np.float64) / np.sqrt(v.astype(np.float64) + EPS))
        bias = b.astype(np.float64) - m.astype(np.float64) * scale
        wf = w.astype(np.float64) * scale.reshape(-1, *([1] * (w.ndim - 1)))
        return wf.astype(np.float32), bias.astype(np.float32)

    w1f, bias1 = fold(w1, g1, b1, m1, v1)   # [256,1024,1,1]
    w2f, bias2 = fold(w2, g2, b2, m2, v2)   # [256,256,3,3]
    w3f, bias3 = fold(w3, g3, b3, m3, v3)   # [1024,256,1,1]

    f8 = ml_dtypes.float8_e4m3
    bf = ml_dtypes.bfloat16

    def q8(a):
        return np.clip(a, -240.0, 240.0).astype(f8)

    # w1 DoubleRow image [p, (kp i m)]: [p, kp, i, m] = w1f[m, (2kp+i)*128+p]
    w1t = np.ascontiguousarray(
        (w1f[:, :, 0, 0] * S1).T.reshape(KP1, 2, P, WIDTH)
        .transpose(2, 0, 1, 3).reshape(P, KP1 * 2 * WIDTH))
    # w2 image [p, (t i m)]: t = dy*3+dx, i = input block
    w2t = np.ascontiguousarray(
        (w2f * (S2 / S1)).transpose(2, 3, 1, 0)
        .reshape(9, KB2, P, WIDTH).transpose(2, 0, 1, 3)
        .reshape(P, 9 * KB2 * WIDTH))
    # w3 image [p, (i m)]
    w3t = np.ascontiguousarray(
        (w3f[:, :, 0, 0] * (S3 / S2)).T.reshape(KB2, P, C_OUT)
        .transpose(1, 0, 2).reshape(P, KB2 * C_OUT))

    b1h = (bias1 * S1).reshape(KB2, P).T                  # [P, 2]
    b2h = (bias2 * S2).reshape(KB2, P).T                  # [P, 2]
    ball = np.ascontiguousarray(
        np.concatenate([b1h, b2h], axis=1), dtype=np.float32)

    # x -> per-core partition-major [P, KB1*NLOC*HW], columns per k-block
    # ordered (pair, j=2r+i, c):
    # [core, pair, i, kb, p, r, c] -> [core, p, kb, pair, r, i, c]
    xs = (x.reshape(NCORES, NPAIRS, 2, KB1, P, 14, 14)
          .transpose(0, 4, 3, 1, 5, 2, 6)
          .reshape(NCORES, P, KB1 * NLOC * HW))
    xq = q8(xs)
    # residual: x + conv3 bias per channel, bf16
    xrf = x + bias3[None, :, None, None]
    xr = (xrf.reshape(NCORES, NPAIRS, 2, KB1, P, 14, 14)
          .transpose(0, 4, 3, 1, 5, 2, 6)
          .reshape(NCORES, P, KB1 * NLOC * HW)).astype(bf)

    ident = (np.eye(P, dtype=np.float32) * S3).astype(bf)

    common = {"w1t": q8(w1t), "w2t": q8(w2t), "w3t": q8(w3t),
              "biases": ball, "ident": ident}
    in_maps = [dict(common,
                    xq=np.ascontiguousarray(xq[i]),
                    xr=np.ascontiguousarray(xr[i]))
               for i in range(NCORES)]
    return in_maps


def kernel(**inputs):
    inputs = {k: np.asarray(v) for k, v in inputs.items()}
    in_maps = _prep(**inputs)
    nc = _build()
    res = run_bass_kernel_spmd(nc, in_maps, core_ids=list(range(NCORES)))

    y = np.empty((NCORES * NLOC, C_OUT, 14, 14), dtype=np.float32)
    for i in range(NCORES):
        r = np.asarray(res.results[i]["y"], dtype=np.float32)  # [MB3,P,N*HW]
        # columns are (pair, j=2r+i, c): [m, p, pair, r, i, c]
        r = (r.reshape(MB3, P, NPAIRS, 14, 2, 14)
             .transpose(2, 4, 0, 1, 3, 5)
             .reshape(NLOC, C_OUT, 14, 14))
        y[i * NLOC:(i + 1) * NLOC] = r
    return y


# revision 24
# speedup vs baseline: 1.0970x; 1.0970x over previous
"""Trainium2 Bass kernel for a ResNet Bottleneck block (inference).

Reference computation (NCHW, N=128, Cin=Cout=1024, width=256, H=W=14):
    out = relu(bn1(conv1x1(x, w1)))          # 1024 -> 256
    out = relu(bn2(conv3x3(out, w2, pad=1))) # 256 -> 256
    out = bn3(conv1x1(out, w3))              # 256 -> 1024
    y   = relu(out + x)

Strategy:
- Data-parallel: batch 128 sharded as 16 images per NeuronCore (8 cores),
  conv/BN params replicated. One NEFF, SPMD via run_bass_kernel_spmd.
- BN folded on host into per-channel weight scale + bias.
- All convs run in fp8-e4m3 with MatmulPerfMode.DoubleRow: each matmul
  contracts K=256 (two 128-channel blocks stacked in AP dim 1) at double
  the bf16 MAC rate. Weights/activations are scaled host-side
  (s1=16, s2=64, s3=256) to sit in e4m3's healthy range; the scale is
  unwound for free: relu(s*a) = s*relu(a), so each conv's input scale is
  folded into the next conv's weights, and the final 1/s3 rides the
  eviction op.
- fp32 PSUM accumulation. conv3's residual is added in PSUM by a bf16
  identity-weight matmul (weights = s3*I) on the bf16 x tiles, so conv3
  eviction is a single relu-and-scale op per group. conv3's BN bias is
  folded into the residual tiles host-side (x + b3).
- Per-image-pair layouts use row-interleaved fields (j = 2*row + img) so
  the 3x3 conv's DoubleRow moving operand is a 4-dim AP
  [p, kpair, 28 interleaved rows, 14 cols] over a zero-padded 32x16
  field.
- PSUM groups are allocated as 2-bank pair tiles [P, 1024] so evictions
  process two groups per DVE/ACT op (halves op count + semaphores).
"""

import sys

if "/opt/trn_rl_repo" not in sys.path:
    sys.path.insert(0, "/opt/trn_rl_repo")

import numpy as np
import ml_dtypes

import concourse.bass as bass
import concourse.bacc as bacc
import concourse.tile as tile
from concourse import mybir
from concourse.bass_utils import run_bass_kernel_spmd

EPS = 1e-5
NCORES = 8
NLOC = 16          # images per core
C_IN = 1024
WIDTH = 256
C_OUT = 1024
HW = 196           # 14*14
P = 128
KB1 = C_IN // P    # 8 input channel blocks
KP1 = KB1 // 2     # 4 DoubleRow channel-block pairs for conv1
KB2 = WIDTH // P   # 2 channel blocks for conv2/conv3 input
MB3 = C_OUT // P   # 8 output channel blocks for conv3
NPAIRS = NLOC // 2  # 8 image pairs; N=392 per matmul
NF = 2 * HW        # 392
FLD = 512          # padded interleaved pair-field: 32 rows x 16 cols
BANK = 512         # PSUM bank, fp32 elements per partition

S1, S2, S3 = 16.0, 64.0, 256.0

F8 = mybir.dt.float8e4
BF16 = mybir.dt.bfloat16
F32 = mybir.dt.float32
Relu = mybir.ActivationFunctionType.Relu
DR = mybir.MatmulPerfMode.DoubleRow

_cached = {}


def _build():
    """Build + compile the SPMD NEFF (one core's program). Cached."""
    if "nc" in _cached:
        return _cached["nc"]

    nc = bacc.Bacc("TRN2", target_bir_lowering=False, debug=False,
                   num_devices=NCORES)

    # x tensors are partition-major in DRAM: per partition one long
    # contiguous run per DMA chunk (best descriptor efficiency)
    xq_d = nc.dram_tensor("xq", [P, KB1 * NLOC * HW], F8,
                          kind="ExternalInput")
    xr_d = nc.dram_tensor("xr", [P, KB1 * NLOC * HW], BF16,
                          kind="ExternalInput")
    w1_d = nc.dram_tensor("w1t", [P, KP1 * 2 * WIDTH], F8,
                          kind="ExternalInput")
    w2_d = nc.dram_tensor("w2t", [P, 9 * KB2 * WIDTH], F8,
                          kind="ExternalInput")
    w3_d = nc.dram_tensor("w3t", [P, KB2 * C_OUT], F8, kind="ExternalInput")
    b_d = nc.dram_tensor("biases", [P, 2 * KB2], F32, kind="ExternalInput")
    id_d = nc.dram_tensor("ident", [P, P], BF16, kind="ExternalInput")
    y_d = nc.dram_tensor("y", [MB3, P, NLOC * HW], BF16, kind="ExternalOutput")

    with tile.TileContext(nc) as tc:
        _emit(tc, nc, xq_d, xr_d, w1_d, w2_d, w3_d, b_d, id_d, y_d)

    nc.compile()
    _cached["nc"] = nc
    return nc


def _emit(tc, nc, xq_d, xr_d, w1_d, w2_d, w3_d, b_d, id_d, y_d):
    """PE-density-oriented emission.

    - Every phase runs its contraction loop OUTER over 8 concurrently-open
      PSUM groups (8 banks via 4 two-bank pair tiles), group index
      innermost, so consecutive matmuls target different banks and
      pipeline at the issue rate (~165ns for N=392 fp8 DoubleRow).
    - Startup: the PE clock needs ~3us of continuous activity to reach
      2.4 GHz, so warm-up matmuls are gated only on a scratch memset that
      is the FIRST gpsimd instruction (before any DMA issue). The xq
      stream gets exclusive DMA bandwidth until it completes; xr/w2/w3
      are chained behind it.
    - Evictions alternate DVE/ACT, one op per PSUM-bank pair.
    """
    import contextlib

    Alu = mybir.AluOpType

    with contextlib.ExitStack() as ctx:
        const = ctx.enter_context(tc.tile_pool(name="const", bufs=1))
        xpool = ctx.enter_context(tc.tile_pool(name="xpool", bufs=1))
        opool = ctx.enter_context(tc.tile_pool(name="opool", bufs=1))
        psp = ctx.enter_context(tc.tile_pool(name="psp", bufs=8, space="PSUM"))
        evp = ctx.enter_context(tc.tile_pool(name="evp", bufs=2))

        from concourse.tile import add_dep_helper

        # PE warm-up first: scratch memset is gpsimd's first instruction,
        # so the dummy matmuls start right after the framework preamble
        # and keep the PE busy (ramping its clock) while x loads.
        # Warm-up across 4 different PSUM banks so the dummy matmuls
        # pipeline at issue rate (same-bank back-to-back matmuls serialize
        # at full round-trip latency and never ramp the clock). Exactly 4:
        # the 5th/6th op measurably stalled ~3us and blocked the PE queue,
        # delaying conv1 behind it.
        scratch = const.tile([P, 512], BF16, name="scratch", tag="scratch")
        nc.gpsimd.memset(scratch[:], 0.0)
        for w in range(4):
            warm_ps = psp.tile([P, 512], F32, name=f"warm_{w}", tag="ps")
            nc.tensor.matmul(warm_ps[:], scratch[:, 0:P], scratch[:],
                             start=True, stop=True)

        # ---- Loads ------------------------------------------------------
        # Three independent hw DMA queues (one per issuing engine):
        # sync + gpsimd stripe the x tensors (both queues in parallel),
        # scalar carries all the small weight/bias loads.
        w1sb = const.tile([P, KP1 * 2 * WIDTH], F8, name="w1sb", tag="w1sb")
        nc.scalar.dma_start(w1sb[:], w1_d.ap())
        w1v = w1sb[:].rearrange("p (j i c) -> p j i c", j=KP1, i=2)

        ball = const.tile([P, 2 * KB2], F32, name="ball", tag="ball")
        nc.scalar.dma_start(ball[:], b_d.ap())
        b1_t = ball[:, 0:KB2]
        b2_t = ball[:, KB2:2 * KB2]

        w2sb = const.tile([P, 9 * KB2 * WIDTH], F8, name="w2sb", tag="w2sb")
        nc.scalar.dma_start(w2sb[:], w2_d.ap())
        w2v = w2sb[:].rearrange("p (t i c) -> p t i c", t=9, i=2)

        w3sb = const.tile([P, KB2 * C_OUT], F8, name="w3sb", tag="w3sb")
        nc.scalar.dma_start(w3sb[:], w3_d.ap())
        w3v = w3sb[:].rearrange("p (i c) -> p i c", i=2)

        id_t = const.tile([P, P], BF16, name="id_t", tag="id_t")
        nc.scalar.dma_start(id_t[:], id_d.ap())

        xq = xpool.tile([P, KB1 * NLOC * HW], F8, name="xq", tag="xq")
        xqv = xq[:].rearrange("p (k n) -> p k n", k=KB1)
        CH = 2 * NLOC * HW  # chunk: 2 channel blocks, contiguous both sides
        xq_dmas = []
        for j in range(KB1 // 2):
            eng = nc.sync if j % 2 == 0 else nc.gpsimd
            i = eng.dma_start(xq[:, j * CH:(j + 1) * CH],
                              xq_d.ap()[:, j * CH:(j + 1) * CH])
            n = len(xq_dmas)
            if n >= 2:
                add_dep_helper(i.ins, xq_dmas[n - 2], reason="xq load pacing")
            xq_dmas.append(i.ins)

        # residual x (bf16, conv3 bias folded in), behind xq on both queues
        xr = xpool.tile([P, KB1 * NLOC * HW], BF16, name="xr", tag="xr")
        xrv = xr[:].rearrange("p (k n) -> p k n", k=KB1)
        xr_dmas = []
        for j in range(KB1 // 2):
            eng = nc.sync if j % 2 == 0 else nc.gpsimd
            i = eng.dma_start(xr[:, j * CH:(j + 1) * CH],
                              xr_d.ap()[:, j * CH:(j + 1) * CH])
            n = len(xr_dmas)
            if n < 2:
                add_dep_helper(i.ins, xq_dmas[2 + n], reason="xr behind xq")
            else:
                add_dep_helper(i.ins, xr_dmas[n - 2], reason="xr load pacing")
            xr_dmas.append(i.ins)

        # conv1 output: zero-padded row-interleaved pair fields, fp8.
        # Per image pair a 32x16 field (j = 2*padrow + img), payload at
        # j in 2..29, cols 1..14. Layout [P, KB2 * NPAIRS * FLD].
        # Only the pad cells are zeroed (3 small memsets, not the full
        # field): top/bottom pad rows and the left/right pad columns.
        out1 = opool.tile([P, KB2 * NPAIRS * FLD], F8, name="out1",
                          tag="out1")
        kq = KB2 * NPAIRS  # 16 fields, stride FLD
        fv = out1[:].rearrange("p (f j c) -> p f j c", f=kq, j=32, c=16)
        nc.vector.memset(fv[:, :, 0:2, :], 0.0)      # top pad rows j=0,1
        nc.vector.memset(fv[:, :, 30:32, :], 0.0)    # bottom pad rows
        nc.vector.memset(fv[:, :, 2:30, 0:1], 0.0)   # left pad col
        nc.vector.memset(fv[:, :, 2:30, 15:16], 0.0)  # right pad col
        out1v = out1[:].rearrange("p (k q j c) -> p k q j c",
                                  k=KB2, q=NPAIRS, j=32, c=16)

        out2 = opool.tile([P, KB2 * NLOC * HW], F8, name="out2", tag="out2")
        out2v = out2[:].rearrange("p (k n) -> p k n", k=KB2)

        # ---- conv1 (1x1, 1024->256) + bias + relu -> padded out1 --------
        # Per np-half: 8 open groups (4 npairs x 2 m), contraction kp outer.
        for half in range(2):
            nps = [half * 4 + j for j in range(4)]
            grp = {}
            for np_ in nps:
                for m in range(KB2):
                    grp[(np_, m)] = psp.tile([P, NF], F32,
                                             name=f"ps1_{np_}_{m}", tag="ps")
            for kp in range(KP1):
                for m in range(KB2):
                    for np_ in nps:
                        nc.tensor.matmul(
                            grp[(np_, m)][:],
                            w1v[:, kp, :, m * P:(m + 1) * P],
                            xqv[:, 2 * kp:2 * kp + 2,
                                np_ * NF:(np_ + 1) * NF],
                            start=(kp == 0), stop=(kp == KP1 - 1),
                            perf_mode=DR,
                        )
            for np_ in nps:
                for m in range(KB2):
                    dst = out1v[:, m, np_, 2:30, 1:15]
                    src = (grp[(np_, m)][:]
                           .rearrange("p (j c) -> p j c", j=28))
                    if np_ % 2 == 1:
                        nc.vector.tensor_scalar(dst, src, b1_t[:, m:m + 1],
                                                0.0, Alu.add, Alu.max)
                    else:
                        nc.scalar.activation(dst, src, Relu,
                                             bias=b1_t[:, m:m + 1])

        # ---- conv2 (3x3, 256->256, pad 1) + bias + relu -> out2 ----------
        # Per np-half: 8 open groups, contraction tap outer (each tap is a
        # DoubleRow pair over the two input channel blocks). The moving
        # operand is the shifted window over the interleaved padded field:
        # rows 2*dy..2*dy+27, cols dx..dx+13.
        for half in range(2):
            nps = [half * 4 + j for j in range(4)]
            grp = {}
            for np_ in nps:
                for m in range(KB2):
                    grp[(np_, m)] = psp.tile([P, NF], F32,
                                             name=f"ps2_{np_}_{m}", tag="ps")
            for t in range(9):
                dy, dx = t // 3, t % 3
                for m in range(KB2):
                    for np_ in nps:
                        rhs = out1v[:, :, np_, 2 * dy:2 * dy + 28,
                                    dx:dx + 14]
                        nc.tensor.matmul(
                            grp[(np_, m)][:]
                            .rearrange("p (j c) -> p j c", j=28),
                            w2v[:, t, :, m * P:(m + 1) * P],
                            rhs,
                            start=(t == 0), stop=(t == 8),
                            perf_mode=DR,
                        )
            for np_ in nps:
                for m in range(KB2):
                    dst = out2v[:, m, np_ * NF:(np_ + 1) * NF]
                    src = grp[(np_, m)][:]
                    if np_ % 2 == 1:
                        nc.vector.tensor_scalar(dst, src, b2_t[:, m:m + 1],
                                                0.0, Alu.add, Alu.max)
                    else:
                        nc.scalar.activation(dst, src, Relu,
                                             bias=b2_t[:, m:m + 1])

        # ---- conv3 (1x1, 256->1024) + residual + relu -> y --------------
        # Per m: 8 open groups as 4 pair tiles. Each group is one DoubleRow
        # matmul (K=256) plus a bf16 identity matmul (weights s3*I) adding
        # s3 * (x + b3) into PSUM; eviction is relu(psum)/s3 per pair.
        inv_s3 = 1.0 / S3
        for m in range(MB3):
            grp = {}
            for np_ in range(NPAIRS):
                grp[np_] = psp.tile([P, NF], F32, name=f"ps3_{np_}", tag="ps")
            for np_ in range(NPAIRS):
                nc.tensor.matmul(
                    grp[np_][:],
                    w3v[:, :, m * P:(m + 1) * P],
                    out2v[:, :, np_ * NF:(np_ + 1) * NF],
                    start=True, stop=False,
                    perf_mode=DR,
                )
            for np_ in range(NPAIRS):
                nc.tensor.matmul(
                    grp[np_][:], id_t[:],
                    xrv[:, m, np_ * NF:(np_ + 1) * NF],
                    start=False, stop=True,
                )
            ystage = evp.tile([P, NLOC * HW], BF16, name="ystage",
                              tag="ystage", bufs=3)
            for np_ in range(NPAIRS):
                dst = ystage[:, np_ * NF:(np_ + 1) * NF]
                if (np_ + m) % 2 == 1:
                    nc.vector.tensor_scalar(dst, grp[np_][:], 0.0, inv_s3,
                                            Alu.max, Alu.mult)
                else:
                    nc.scalar.activation(dst, grp[np_][:], Relu,
                                         bias=0.0, scale=inv_s3)
            nchunk = 4 if m == MB3 - 1 else 2
            CNF = NLOC * HW // nchunk
            for c in range(nchunk):
                eng = nc.sync if (m * 2 + c) % 2 == 0 else nc.gpsimd
                eng.dma_start(y_d.ap()[m][:, c * CNF:(c + 1) * CNF],
                              ystage[:, c * CNF:(c + 1) * CNF])


def _prep(x, w1, g1, b1, m1, v1, w2, g2, b2, m2, v2, w3, g3, b3, m3, v3):
    """Host-side: fold BN, scale + quantize weights to fp8, arrange SBUF
    images, shard + interleave x."""
    def fold(w, g, b, m, v):
        scale = (g.astype(np.float64) / np.sqrt(v.astype(np.float64) + EPS))
        bias = b.astype(np.float64) - m.astype(np.float64) * scale
        wf = w.astype(np.float64) * scale.reshape(-1, *([1] * (w.ndim - 1)))
        return wf.astype(np.float32), bias.astype(np.float32)

    w1f, bias1 = fold(w1, g1, b1, m1, v1)   # [256,1024,1,1]
    w2f, bias2 = fold(w2, g2, b2, m2, v2)   # [256,256,3,3]
    w3f, bias3 = fold(w3, g3, b3, m3, v3)   # [1024,256,1,1]

    f8 = ml_dtypes.float8_e4m3
    bf = ml_dtypes.bfloat16

    def q8(a):
        return np.clip(a, -240.0, 240.0).astype(f8)

    # w1 DoubleRow image [p, (kp i m)]: [p, kp, i, m] = w1f[m, (2kp+i)*128+p]
    w1t = np.ascontiguousarray(
        (w1f[:, :, 0, 0] * S1).T.reshape(KP1, 2, P, WIDTH)
        .transpose(2, 0, 1, 3).reshape(P, KP1 * 2 * WIDTH))
    # w2 image [p, (t i m)]: t = dy*3+dx, i = input block
    w2t = np.ascontiguousarray(
        (w2f * (S2 / S1)).transpose(2, 3, 1, 0)
        .reshape(9, KB2, P, WIDTH).transpose(2, 0, 1, 3)
        .reshape(P, 9 * KB2 * WIDTH))
    # w3 image [p, (i m)]
    w3t = np.ascontiguousarray(
        (w3f[:, :, 0, 0] * (S3 / S2)).T.reshape(KB2, P, C_OUT)
        .transpose(1, 0, 2).reshape(P, KB2 * C_OUT))

    b1h = (bias1 * S1).reshape(KB2, P).T                  # [P, 2]
    b2h = (bias2 * S2).reshape(KB2, P).T                  # [P, 2]
    ball = np.ascontiguousarray(
        np.concatenate([b1h, b2h], axis=1), dtype=np.float32)

    # x -> per-core partition-major [P, KB1*NLOC*HW], columns per k-block
    # ordered (pair, j=2r+i, c):
    # [core, pair, i, kb, p, r, c] -> [core, p, kb, pair, r, i, c]
    xs = (x.reshape(NCORES, NPAIRS, 2, KB1, P, 14, 14)
          .transpose(0, 4, 3, 1, 5, 2, 6)
          .reshape(NCORES, P, KB1 * NLOC * HW))
    xq = q8(xs)
    # residual: x + conv3 bias per channel, bf16
    xrf = x + bias3[None, :, None, None]
    xr = (xrf.reshape(NCORES, NPAIRS, 2, KB1, P, 14, 14)
          .transpose(0, 4, 3, 1, 5, 2, 6)
          .reshape(NCORES, P, KB1 * NLOC * HW)).astype(bf)

    ident = (np.eye(P, dtype=np.float32) * S3).astype(bf)

    common = {"w1t": q8(w1t), "w2t": q8(w2t), "w3t": q8(w3t),
              "biases": ball, "ident": ident}
    in_maps = [dict(common,
                    xq=np.ascontiguousarray(xq[i]),
                    xr=np.ascontiguousarray(xr[i]))
               for i in range(NCORES)]
    return in_maps


def kernel(**inputs):
    inputs = {k: np.asarray(v) for k, v in inputs.items()}
    in_maps = _prep(**inputs)
    nc = _build()
    res = run_bass_kernel_spmd(nc, in_maps, core_ids=list(range(NCORES)))

    y = np.empty((NCORES * NLOC, C_OUT, 14, 14), dtype=np.float32)
    for i in range(NCORES):
        r = np.asarray(res.results[i]["y"], dtype=np.float32)  # [MB3,P,N*HW]
        # columns are (pair, j=2r+i, c): [m, p, pair, r, i, c]
        r = (r.reshape(MB3, P, NPAIRS, 14, 2, 14)
             .transpose(2, 4, 0, 1, 3, 5)
             .reshape(NLOC, C_OUT, 14, 14))
        y[i * NLOC:(i + 1) * NLOC] = r
    return y
